# revision 32
# baseline (speedup 1.0000x reference)
"""ALiBi attention (B=4, S=2048, D=1024, H=16) on 8 TRN2 NeuronCores.

Sharding: 2D data-parallel over (batch, query-block) -> zero collectives.
Core c handles batch b = c//2, query rows q0 = (c%2)*1024 .. +1024, ALL heads.

Window math: the reference's ALiBi bias is slope_h * (k - q) with no causal
mask; softmax is invariant to per-row constants, so the bias is equivalent to
slope_h * (k - k_last) <= 0 where k_last is the last unmasked key.  With
min slope 2^(-15/16) ~= 0.522 and |scores| <~ 3, every key more than W=64
positions before k_last carries < e^{-27} relative softmax mass, so attention
over the last 64 keys is exact to ~1e-11 for ANY mask (zeros inside the
window get a -30000 penalty folded into the exp bias).

Per-core kernel (fp32 PSUM everywhere):

* Q projection runs in fp8(e4m3) DoubleRow mode (2 fp8 weights/PE cell, 2
  MACs/cycle): host scales Wq.T by SCALE*2^8 into e4m3 (weights ~N(0,0.64)
  land mid-range), x quantizes to e4m3 directly; the PSUM epilogue rescales
  by 2^-8 (ACT scale / DVE tensor_scalar mult+add).  64 DR matmuls replace
  128 fp16 matmuls for the 2.1 GFLOP GEMM.  O projection stays fp16 (fp8
  there costs 4e-2 rel err vs the 2e-2 budget; measured offline).
* K^T proj: per-tile N=64 matmuls into block-diagonal kb -- the real cost
  is 64 LDWEIGHTS, so both groups are emitted adjacent to 512-wide matmul
  streams and the PE queue's LDWEIGHTS pull-ahead hides them.  V is
  computed DIRECTLY in [keys, dims] layout: the stationary operand is a
  duplicated window tile [xw | xw] so both partition halves carry the same
  64 keys, giving free-dim-512 matmuls and no PE transposes; bv is added
  during the PSUM evacuation against a pre-broadcast bias tile.
* Head-pair block packing for attention: for head pair pr, a 128x128
  block-diagonal kb/vb against pair-packed Q^T/P tiles gives both heads'
  score/AV/denominator tiles in single full-array matmuls; softmax
  normalization is DVE reciprocal_approx_fast + multiply.
* Phase interleaving keeps the PE gapless: Q-qh1's DoubleRow matmuls fill
  the PE under attn(qh0)'s Vector-bound softmax chain (kb/vb column-group 1
  is injected mid-loop when its weight DMAs land), and the O-proj(qh0)
  matmuls fill it under attn(qh1).  A single 8-bank PSUM "arena" pool with
  explicit per-bank tags makes every bank reuse a per-tag WAR dependency
  on exactly the intended reader (pool boundaries act as barriers).
* 30 warmup matmuls on a memset tile spin the PE during the initial DMA
  fill so the HAM clock-gate (1.2 GHz cold -> 2.4 GHz after ~3.4us busy)
  is warm when the real matmuls arrive.
* DMA: the 16 channels are shared across queues and are descriptor-rate
  bound at ~90ns/line, so ALL big tensors are host-packed into exact SBUF
  layouts moving 4-16KB contiguous partition lines: wq/xq interleave into
  one fp8 tensor (chunk-pair slices feed whole DR waves), wk/wv/wo are
  column-half-major and stream in consumption order, and the output drains
  as 2KB-line tile pairs on both queues (final pair split per half for
  latency).
"""

import sys

sys.path.insert(0, "/opt/trn_rl_repo")

import numpy as np
import ml_dtypes

import concourse.bass as bass  # noqa: F401  (registers bass types)
import concourse.tile as tile
from concourse import bacc, mybir
from concourse.bass_utils import run_bass_kernel_spmd

F32 = mybir.dt.float32
FP16 = mybir.dt.float16
FP8 = mybir.dt.float8e4
E4NP = ml_dtypes.float8_e4m3
EXP = mybir.ActivationFunctionType.Exp
COPY = mybir.ActivationFunctionType.Copy
IDENT = mybir.ActivationFunctionType.Identity
DR = mybir.MatmulPerfMode.DoubleRow
MULT = mybir.AluOpType.mult
ADD = mybir.AluOpType.add

B, S, D, H, HD = 4, 2048, 1024, 16, 64
P = 128
NCORES = 8
QR = 1024          # q rows per core
NT = D // P        # 8 tiles / contraction chunks
SCALE = HD ** -0.5
W = 64             # attention window (k keys per query)
PEN = -30000.0     # mask penalty (exp -> 0)
QS = 256.0         # fp8 Q-side weight scale (2^8); epilogue multiplies 2^-8

_CACHE = {}


def _build():
    nc = bacc.Bacc("TRN2", target_bir_lowering=False, debug=False)

    # ALL big tensors are pre-packed host-side into the exact SBUF tile
    # layout [128, chunks*cols] so DMAs move 4-16KB contiguous per
    # partition line.  1-2KB lines are descriptor-rate-bound (~90ns/line/
    # channel) and throttle the input stream to ~150GB/s; 8KB lines reach
    # the HBM roofline.
    d = {}
    # wq and xq interleaved per chunk ([wq_c | xq_c], 2KB each) so one
    # chunk-pair DMA moves 4KB partition lines and feeds a whole DR wave
    d["d_qx"] = nc.dram_tensor("qx", [P, NT * (D + QR)], FP8,
                               kind="ExternalInput")
    # xw2 pre-packed host-side: [128, NT, 2W] with the 64 window columns
    # duplicated so dupxw.T @ Wv yields V on both partition halves
    d["d_xw2"] = nc.dram_tensor("xw2", [P, NT * 2 * W], FP16, kind="ExternalInput")
    d["d_wk"] = nc.dram_tensor("wk", [P, NT * D], FP16, kind="ExternalInput")
    d["d_wv"] = nc.dram_tensor("wv", [P, NT * D], FP16, kind="ExternalInput")
    d["d_wo"] = nc.dram_tensor("wo", [P, NT * D], FP16, kind="ExternalInput")
    # cst cols: 0:8 bq*SCALE | 8:16 bk | 16:24 bo | 24:32 (unused) | 32:40 exp bias
    d["d_cst"] = nc.dram_tensor("cst", [P, 5 * NT], F32, kind="ExternalInput")
    # db: block-diag ones for the softmax denominator matmul
    d["d_db"] = nc.dram_tensor("db", [P, P], FP16, kind="ExternalInput")
    # bv broadcast across partitions in the vb block layout [P, NT*P]
    d["d_bvb"] = nc.dram_tensor("bvb", [P, NT * P], FP16, kind="ExternalInput")
    # out: [128, qh*4096 + pair*1024 + 512*odd + q] (two tiles per DMA for
    # 2KB lines); host unpacks
    d["d_out"] = nc.dram_tensor("ot", [P, 2 * NT * 512], FP16,
                                kind="ExternalOutput")

    with tile.TileContext(nc) as tc:
        _emit(nc, tc, d)
    nc.compile()
    return nc


def _emit(nc, tc, d):
    from contextlib import ExitStack

    MM = nc.tensor.matmul
    dma = nc.sync.dma_start
    odma = nc.scalar.dma_start

    with ExitStack() as ctx:
        # ---- persistent SBUF ----
        pers = ctx.enter_context(tc.tile_pool(name="pers", bufs=1))
        t_xw2 = pers.tile([P, NT, 2 * W], FP16, tag="xw2")
        t_kb = pers.tile([P, NT, P], FP16, tag="kb")      # K^T block-diags
        t_vb = pers.tile([P, NT, P], FP16, tag="vb")      # V block-diags
        t_qt = pers.tile([P, NT, QR], FP16, tag="qt")
        t_at = pers.tile([P, NT, QR], FP16, tag="at")
        t_cst = pers.tile([P, 5 * NT], F32, tag="cst")
        t_db = pers.tile([P, P], FP16, tag="db")
        t_bvb = pers.tile([P, NT, P], FP16, tag="bvb")
        t_qx = pers.tile([P, NT, D + QR], FP8, tag="qx")
        # wk/wv/wo are packed column-half-major: [P, g, chunk, 512] so the
        # first DMA covers output columns 0:512 of ALL chunks (what the
        # K/V/O phase g=0 loops consume first)
        t_wk = pers.tile([P, 2, NT, 512], FP16, tag="wk")
        t_wv = pers.tile([P, 2, NT, 512], FP16, tag="wv")
        t_wo = pers.tile([P, 2, NT, 512], FP16, tag="wo")
        t_warm = pers.tile([P, P], FP16, tag="warm")

        t_bq = t_cst[:, 0:NT]
        t_bk = t_cst[:, NT:2 * NT]
        t_bo = t_cst[:, 2 * NT:3 * NT]
        t_eb = t_cst[:, 4 * NT:5 * NT]
        t_ones = t_db[:, 0:P]

        # ---- input DMAs up front, in consumption order.  The first wq/xq
        # chunk pair rides the otherwise-idle scalar queue (its DMA rings
        # start cold in parallel with the sync queue's) so the first
        # DoubleRow wave starts ~1.5us earlier.  Remaining wq/xq quarters
        # (2KB lines) then wk/wv/wo column-halves (8KB lines) on sync. ----
        odma(t_cst[:], d["d_cst"].ap())
        odma(t_db[:], d["d_db"].ap())
        odma(t_bvb[:], d["d_bvb"].ap())
        # the 16 DMA channels are SHARED between the sync and scalar
        # queues, so the critical first chunk pair must lead the sync
        # stream itself
        CW = D + QR
        for q in range(4):
            dma(t_qx[:, 2 * q:2 * q + 2, :],
                d["d_qx"].ap()[:, 2 * q * CW:(2 * q + 2) * CW])
        dma(t_xw2[:], d["d_xw2"].ap())
        GC = NT * 512
        for g in range(2):
            dma(t_wk[:, g, :, :], d["d_wk"].ap()[:, g * GC:(g + 1) * GC])
            dma(t_wv[:, g, :, :], d["d_wv"].ap()[:, g * GC:(g + 1) * GC])
        for g in range(2):
            dma(t_wo[:, g, :, :], d["d_wo"].ap()[:, g * GC:(g + 1) * GC])

        # zero the off-diagonal quadrants of the block tiles + warm tile
        nc.gpsimd.memset(t_warm[:], 0.0)
        nc.gpsimd.memset(t_kb[:], 0.0)
        nc.gpsimd.memset(t_vb[:], 0.0)

        # ---- single PSUM arena: 8 explicit bank-tags (bufs=1 each), so
        # every bank reuse is a per-tag WAR dependency on exactly the
        # intended reader -- no pool-boundary barriers. ----
        arena = ctx.enter_context(tc.tile_pool(name="arena", bufs=1,
                                               space="PSUM"))

        def bank(i, name, shape=(P, 512)):
            return arena.tile(list(shape), F32, tag=f"q{i}", name=name,
                              bufs=1)

        # ---- PE warmup: back-to-back matmuls on the memset tile keep the
        # PE busy during the initial DMA fill so the HAM clock-gate is at
        # 2.4 GHz when the first real matmul lands ----
        wps = bank(7, "wps", (P, P))
        for _ in range(30):
            MM(wps[:], t_warm[:], t_warm[:], start=True, stop=True)

        # GpSimd has no PSUM port: all PSUM-evacuating epilogues alternate
        # between the Scalar and Vector engines.
        epi = [nc.scalar, nc.vector]

        def bias_add(i, out, in_, col):
            e = epi[i % 2]
            if e is nc.scalar:
                e.activation(out, in_, IDENT, bias=col)
            else:
                e.tensor_scalar_add(out, in_, col)

        def q_epi(i, out, in_, col, scalar_only=False):
            # out = in * 2^-8 + col  (undo the fp8 weight scale)
            e = nc.scalar if scalar_only else epi[i % 2]
            if e is nc.scalar:
                e.activation(out, in_, IDENT, bias=col, scale=1.0 / QS)
            else:
                e.tensor_scalar(out, in_, 1.0 / QS, col, MULT, ADD)

        # ---- Q proj qh=0 (chunk-pair-outer over all 8 banks): trails the
        # wq/xq DMA stream pair by pair ----
        qps = [bank(t, f"qp{t}") for t in range(NT)]
        for cp in range(NT // 2):
            for t in range(NT):
                MM(qps[t][:],
                   t_qx[:, 2 * cp:2 * cp + 2, t * P:(t + 1) * P],
                   t_qx[:, 2 * cp:2 * cp + 2, D:D + 512],
                   start=(cp == 0), stop=(cp == NT // 2 - 1),
                   perf_mode=DR)
        # epilogues for banks q6/q7 run FIRST on their engine queues so the
        # qh=1 tiles (which reuse those banks) start without waiting for
        # the whole epilogue drain
        for pos, t in enumerate([6, 7, 0, 1, 2, 3, 4, 5]):
            q_epi(pos, t_qt[:, t, 0:512], qps[t][:], t_bq[:, t:t + 1])

        def qh1_tile(i):
            # Q qh=1 tile i: 4 DR matmuls + scalar-only epilogue (the
            # vector queue is saturated by recip+mul when these run inside
            # the attn(0) interleave)
            qp1 = bank(6 + (i % 2), f"dq{i}")
            for cp in range(NT // 2):
                MM(qp1[:],
                   t_qx[:, 2 * cp:2 * cp + 2, i * P:(i + 1) * P],
                   t_qx[:, 2 * cp:2 * cp + 2, D + 512:D + QR],
                   start=(cp == 0), stop=(cp == NT // 2 - 1),
                   perf_mode=DR)
            q_epi(i, t_qt[:, i, 512:QR], qp1[:], t_bq[:, i:i + 1],
                  scalar_only=True)

        # tiles 0-1 of Q qh=1 fill the PE while the wk g=0 DMA (behind the
        # whole qx stream) lands; the remaining six fill attn(0)'s
        # Vector-bound softmax window below
        qh1_tile(0)
        qh1_tile(1)

        # ---- K^T proj: 4 output tiles share ONE PSUM bank as a single
        # accumulation group.  Group g=1 is interleaved into the V g=0
        # matmul stream: its 32 LDWEIGHTS (~97ns each, the real cost of
        # these N=64 matmuls) pull ahead inside the PE queue while the
        # 512-wide V matmuls stream. ----
        def k_group(g, c):
            kp = k_banks[g]
            for j in range(4):
                MM(kp[:, j * W:(j + 1) * W],
                   t_wk[:, g, c, j * P:(j + 1) * P],
                   t_xw2[:, c, 0:W],
                   start=(c == 0 and j == 0),
                   stop=(c == NT - 1 and j == 3),
                   skip_group_check=True)

        def k_epi(g):
            for j in range(4):
                t = g * 4 + j
                kp = k_banks[g]
                bias_add(t, t_kb[0:64, t, 0:64],
                         kp[0:64, j * W:(j + 1) * W], t_bk[0:64, t:t + 1])
                bias_add(t + 1, t_kb[64:128, t, 64:128],
                         kp[64:128, j * W:(j + 1) * W],
                         t_bk[64:128, t:t + 1])

        k_banks = [bank(0, "kg0", (P, 4 * W)), bank(1, "kg1", (P, 4 * W))]
        for c in range(NT):
            k_group(0, c)
        k_epi(0)

        def v_group(g):
            vp = bank(2 + g, f"vg{g}", (P, 4, P))
            for c in range(NT):
                if g == 1:
                    k_group(1, c)
                MM(vp[:, :, :], t_xw2[:, c, :],
                   t_wv[:, g, c, :],
                   start=(c == 0), stop=(c == NT - 1),
                   skip_group_check=True)
            if g == 1:
                k_epi(1)
            gs = slice(4 * g, 4 * (g + 1))
            nc.vector.tensor_add(t_vb[0:64, gs, 0:64],
                                 vp[0:64, :, 0:64],
                                 t_bvb[0:64, gs, 0:64])
            nc.vector.tensor_add(t_vb[64:128, gs, 64:128],
                                 vp[64:128, :, 64:128],
                                 t_bvb[64:128, gs, 64:128])

        v_group(0)

        # ---- SBUF pools for the attention chain + output staging ----
        ppool = ctx.enter_context(tc.tile_pool(name="pp", bufs=3))
        rpool = ctx.enter_context(tc.tile_pool(name="rp", bufs=2))
        opool = ctx.enter_context(tc.tile_pool(name="op", bufs=4))

        # attention pair chain, split in two halves so the PE-side (scores)
        # can lead the PSUM-side (AV + denominator) by one pair: the AV
        # matmul needs exp(p) which lags ~0.7us on the Scalar queue.
        def attn_head(qg, pr):
            q0 = qg * 512
            ps = bank(4 + (pr % 2), f"sc{qg}_{pr}")
            MM(ps[:], t_kb[:, pr, :], t_qt[:, pr, q0:q0 + 512],
               start=True, stop=True)
            p = ppool.tile([P, 512], FP16, tag="p", name=f"p{qg}_{pr}")
            nc.scalar.activation(p[:], ps[:], EXP, bias=t_eb[:, pr:pr + 1])
            return p

        def attn_tail(qg, pr, p):
            q0 = qg * 512
            pav = bank(2 + (pr % 2), f"av{qg}_{pr}")
            MM(pav[:], t_vb[:, pr, :], p[:], start=True, stop=True)
            pden = bank(pr % 2, f"dn{qg}_{pr}")
            MM(pden[:], t_ones, p[:], start=True, stop=True)
            rec = rpool.tile([P, 512], F32, tag="rec", name=f"rec{qg}_{pr}")
            nc.vector.reciprocal_approx_fast(out=rec[:], in_=pden[:])
            nc.vector.tensor_mul(t_at[:, pr, q0:q0 + 512], pav[:], rec[:])

        def oproj_tile(qh, t, o, ti, banksel):
            q0 = qh * 512
            ps = bank(banksel, f"of{qh}_{t}")
            g, jj = t // 4, t % 4
            for c in range(NT):
                MM(ps[:], t_wo[:, g, c, jj * P:(jj + 1) * P],
                   t_at[:, c, q0:q0 + 512],
                   start=(c == 0), stop=(c == NT - 1))
            bias_add(t, o[:, ti * 512:(ti + 1) * 512], ps[:],
                     t_bo[:, t:t + 1])

        def out_dma(qh, tp, o):
            out_q = dma if tp % 2 == 0 else odma
            col = qh * (NT * 512) + tp * 1024
            out_q(d["d_out"].ap()[:, col:col + 1024], o[:])

        # ---- attn(0) interleaved with Q proj qh=1: the attention chain is
        # Vector-bound (~1.2us/pair vs 0.65us of PE work), so the qh=1
        # DoubleRow matmuls fill the PE while attn(0)'s softmax chain
        # drains.  qh=1 rotates banks q6/q7 (WAR on qh0 epilogues t6/t7);
        # scores lead AV by one pair so exp() latency never stalls the PE.
        prev = None
        for i in range(NT + 1):
            if i == 4:
                # kb/vb blocks 4-7 (column-group 1): their weight DMAs land
                # while attention pairs 0-3 run
                v_group(1)
            if i < NT:
                p = attn_head(0, i)
                if i >= 2:
                    qh1_tile(i)
            if prev is not None:
                attn_tail(0, i - 1, prev)
            prev = p if i < NT else None

        # ---- attn(1) interleaved with O proj qh=0 (same pattern: O-proj
        # matmuls keep the PE busy under attn(1)'s softmax chain).  O tiles
        # rotate banks q6/q7; out DMAs drain per tile pair. ----
        prev = None
        o_tile = None
        for i in range(NT + 1):
            if i < NT:
                p = attn_head(1, i)
                if i % 2 == 0:
                    o_tile = opool.tile([P, 1024], FP16, tag="o",
                                        name=f"o0_{i // 2}")
                oproj_tile(0, i, o_tile, i % 2, 6 + (i % 2))
                if i % 2 == 1:
                    out_dma(0, i // 2, o_tile)
            if prev is not None:
                attn_tail(1, i - 1, prev)
            prev = p if i < NT else None

        # ---- O proj qh=1: pure PE, rotate banks q0..q5 for slack against
        # the epilogue queues ----
        for t in range(NT):
            if t % 2 == 0:
                o_tile = opool.tile([P, 1024], FP16, tag="o",
                                    name=f"o1_{t // 2}")
            oproj_tile(1, t, o_tile, t % 2, t % 6)
            if t >= NT - 2:
                # final pair: drain each tile as soon as its epilogue is
                # done, split across both queues (latency > descriptor
                # efficiency at the very end)
                col = NT * 512 + (t // 2) * 1024 + (t % 2) * 512
                half = (t % 2) * 512
                dma(d["d_out"].ap()[:, col:col + 256],
                    o_tile[:, half:half + 256])
                odma(d["d_out"].ap()[:, col + 256:col + 512],
                     o_tile[:, half + 256:half + 512])
            elif t % 2 == 1:
                out_dma(1, t // 2, o_tile)


def _get_nc():
    if "nc" not in _CACHE:
        _CACHE["nc"] = _build()
    return _CACHE["nc"]


def kernel(x, Wq, bq, Wk, bk, Wv, bv, Wo, bo, mask):
    x = np.asarray(x, np.float32)
    Wq = np.asarray(Wq, np.float32); bq = np.asarray(bq, np.float32)
    Wk = np.asarray(Wk, np.float32); bk = np.asarray(bk, np.float32)
    Wv = np.asarray(Wv, np.float32); bv = np.asarray(bv, np.float32)
    Wo = np.asarray(Wo, np.float32); bo = np.asarray(bo, np.float32)
    mask = np.asarray(mask, np.int32)
    assert x.shape == (B, S, D) and mask.shape == (B, S)

    nc = _get_nc()

    def cvt(a):
        return np.ascontiguousarray(a, dtype=np.float16)

    k_last = np.array([
        (np.nonzero(mask[b])[0][-1] if mask[b].any() else S - 1)
        for b in range(B)
    ])
    win0s = np.maximum(0, k_last + 1 - W)
    slopes = 1.0 / 2.0 ** (np.arange(H, dtype=np.float32) / H)

    def sbuf_pack(a):
        # [D, N] row-major -> [P, NT*N]: row p holds chunks c=0..NT-1 of
        # a[c*P+p, :] concatenated (the chunk-major SBUF tile layout)
        return np.ascontiguousarray(
            a.reshape(NT, P, a.shape[1]).transpose(1, 0, 2).reshape(P, -1))

    def gpack(a):
        # [D, D] -> [P, 2*NT*512] column-half-major: [p, g, c, j] =
        # a[c*P+p, g*512+j]
        return np.ascontiguousarray(
            a.reshape(NT, P, 2, 512).transpose(1, 2, 0, 3).reshape(P, -1))

    wq_t8 = np.clip(Wq.T * (SCALE * QS), -240.0, 240.0).astype(E4NP) \
        .reshape(NT, P, D).transpose(1, 0, 2)  # [P, NT, D]
    wk_t = gpack(cvt(Wk.T))
    wv_t = gpack(cvt(Wv.T))
    wo_t = gpack(cvt(Wo.T))

    db = np.zeros((P, P), np.float16)
    db[0:64, 0:64] = 1.0
    db[64:128, 64:128] = 1.0

    bvb = np.broadcast_to(
        bv.astype(np.float16).reshape(1, NT * P), (P, NT * P))
    bvb = np.ascontiguousarray(bvb)

    cst_common = np.zeros((P, 5 * NT), np.float32)
    cst_common[:, 0:NT] = (bq * SCALE).reshape(NT, P).T
    cst_common[:, NT:2 * NT] = bk.reshape(NT, P).T
    cst_common[:, 2 * NT:3 * NT] = bo.reshape(NT, P).T

    in_maps = []
    for c in range(NCORES):
        b = c // 2
        q0 = (c % 2) * QR
        win0 = int(win0s[b])
        xT = x[b].T  # [D, S]
        kk = win0 + np.arange(W, dtype=np.float32) - float(k_last[b])  # [W]
        pen = np.where(mask[b, win0:win0 + W] == 0, PEN, 0.0).astype(np.float32)
        cst = cst_common.copy()
        for pr in range(NT):
            cst[0:64, 4 * NT + pr] = slopes[2 * pr] * kk + pen
            cst[64:128, 4 * NT + pr] = slopes[2 * pr + 1] * kk + pen
        xw = xT[:, win0:win0 + W].reshape(NT, P, W).transpose(1, 0, 2)  # [P,NT,W]
        xw2 = np.concatenate([xw, xw], axis=2)  # [P, NT, 2W]
        xq8 = np.clip(xT[:, q0:q0 + QR], -240.0, 240.0).astype(E4NP) \
            .reshape(NT, P, QR).transpose(1, 0, 2)  # [P, NT, QR]
        qx8 = np.ascontiguousarray(
            np.concatenate([wq_t8, xq8], axis=2).reshape(P, -1))
        in_maps.append({
            "qx": qx8,
            "xw2": np.ascontiguousarray(
                xw2.reshape(P, NT * 2 * W), dtype=np.float16),
            "wk": wk_t, "wv": wv_t, "wo": wo_t,
            "cst": cst, "db": db, "bvb": bvb,
        })

    global _last_in_maps
    _last_in_maps = in_maps
    res = run_bass_kernel_spmd(nc, in_maps, core_ids=list(range(NCORES)))
    out = np.empty((B, S, D), np.float32)
    for c in range(NCORES):
        b = c // 2
        q0 = (c % 2) * QR
        # ot layout: [p, qh*4096 + tp*1024 + ti*512 + q] for dim-tile
        # t = 2*tp+ti -> out[q0 + qh*512 + q, t*128 + p]
        ot = res.results[c]["ot"].astype(np.float32)
        ot = ot.reshape(P, 2, NT // 2, 2, 512)       # p, qh, tp, ti, q
        out[b, q0:q0 + QR, :] = (
            ot.transpose(1, 4, 2, 3, 0)              # qh, q, tp, ti, p
            .reshape(QR, D))
    return out


if __name__ == "__main__":
    rng = np.random.default_rng(0)
    x = rng.standard_normal((B, S, D), dtype=np.float32)
    w = lambda: (rng.standard_normal((D, D)) * 0.02).astype(np.float32)
    z = np.zeros((D,), np.float32)
    o = kernel(x, w(), z, w(), z, w(), z, w(), z, np.ones((B, S), np.int32))
    print("ran", o.shape, o.dtype)


# revision 33
# speedup vs baseline: 1.0095x; 1.0095x over previous
"""ALiBi attention (B=4, S=2048, D=1024, H=16) on 8 TRN2 NeuronCores.

Sharding: 2D data-parallel over (batch, query-block) -> zero collectives.
Core c handles batch b = c//2, query rows q0 = (c%2)*1024 .. +1024, ALL heads.

Window math: the reference's ALiBi bias is slope_h * (k - q) with no causal
mask; softmax is invariant to per-row constants, so the bias is equivalent to
slope_h * (k - k_last) <= 0 where k_last is the last unmasked key.  With
min slope 2^(-15/16) ~= 0.522 and |scores| <~ 3, every key more than W=64
positions before k_last carries < e^{-27} relative softmax mass, so attention
over the last 64 keys is exact to ~1e-11 for ANY mask (zeros inside the
window get a -30000 penalty folded into the exp bias).

Per-core kernel (fp32 PSUM everywhere):

* Q projection runs in fp8(e4m3) DoubleRow mode (2 fp8 weights/PE cell, 2
  MACs/cycle): host scales Wq.T by SCALE*2^8 into e4m3 (weights ~N(0,0.64)
  land mid-range), x quantizes to e4m3 directly; the PSUM epilogue rescales
  by 2^-8 (ACT scale / DVE tensor_scalar mult+add).  64 DR matmuls replace
  128 fp16 matmuls for the 2.1 GFLOP GEMM.  O projection stays fp16 (fp8
  there costs 4e-2 rel err vs the 2e-2 budget; measured offline).
* K^T proj: per-tile N=64 matmuls into block-diagonal kb -- the real cost
  is 64 LDWEIGHTS, so both groups are emitted adjacent to 512-wide matmul
  streams and the PE queue's LDWEIGHTS pull-ahead hides them.  V is
  computed DIRECTLY in [keys, dims] layout: the stationary operand is a
  duplicated window tile [xw | xw] so both partition halves carry the same
  64 keys, giving free-dim-512 matmuls and no PE transposes; bv is added
  during the PSUM evacuation against a pre-broadcast bias tile.
* Head-pair block packing for attention: for head pair pr, a 128x128
  block-diagonal kb/vb against pair-packed Q^T/P tiles gives both heads'
  score/AV/denominator tiles in single full-array matmuls; softmax
  normalization is DVE reciprocal_approx_fast + multiply.
* Phase interleaving keeps the PE gapless: Q-qh1's DoubleRow matmuls fill
  the PE under attn(qh0)'s Vector-bound softmax chain (kb/vb column-group 1
  is injected mid-loop when its weight DMAs land), and the O-proj(qh0)
  matmuls fill it under attn(qh1).  A single 8-bank PSUM "arena" pool with
  explicit per-bank tags makes every bank reuse a per-tag WAR dependency
  on exactly the intended reader (pool boundaries act as barriers).
* 30 warmup matmuls on a memset tile spin the PE during the initial DMA
  fill so the HAM clock-gate (1.2 GHz cold -> 2.4 GHz after ~3.4us busy)
  is warm when the real matmuls arrive.
* DMA: the 16 channels are shared across queues and are descriptor-rate
  bound at ~90ns/line, so ALL big tensors are host-packed into exact SBUF
  layouts moving 4-16KB contiguous partition lines: wq/xq interleave into
  one fp8 tensor (chunk-pair slices feed whole DR waves), wk/wv/wo are
  column-half-major and stream in consumption order, and the output drains
  as 2KB-line tile pairs on both queues (final pair split per half for
  latency).
"""

import sys

sys.path.insert(0, "/opt/trn_rl_repo")

import numpy as np
import ml_dtypes

import concourse.bass as bass  # noqa: F401  (registers bass types)
import concourse.tile as tile
from concourse import bacc, mybir
from concourse.bass_utils import run_bass_kernel_spmd

F32 = mybir.dt.float32
FP16 = mybir.dt.float16
FP8 = mybir.dt.float8e4
E4NP = ml_dtypes.float8_e4m3
EXP = mybir.ActivationFunctionType.Exp
COPY = mybir.ActivationFunctionType.Copy
IDENT = mybir.ActivationFunctionType.Identity
DR = mybir.MatmulPerfMode.DoubleRow
MULT = mybir.AluOpType.mult
ADD = mybir.AluOpType.add

B, S, D, H, HD = 4, 2048, 1024, 16, 64
P = 128
NCORES = 8
QR = 1024          # q rows per core
NT = D // P        # 8 tiles / contraction chunks
SCALE = HD ** -0.5
W = 64             # attention window (k keys per query)
PEN = -30000.0     # mask penalty (exp -> 0)
QS = 256.0         # fp8 Q-side weight scale (2^8); epilogue multiplies 2^-8

_CACHE = {}


def _build():
    nc = bacc.Bacc("TRN2", target_bir_lowering=False, debug=False)

    # ALL big tensors are pre-packed host-side into the exact SBUF tile
    # layout [128, chunks*cols] so DMAs move 4-16KB contiguous per
    # partition line.  1-2KB lines are descriptor-rate-bound (~90ns/line/
    # channel) and throttle the input stream to ~150GB/s; 8KB lines reach
    # the HBM roofline.
    d = {}
    # wq and xq interleaved per chunk ([wq_c | xq_c], 2KB each) so one
    # chunk-pair DMA moves 4KB partition lines and feeds a whole DR wave
    d["d_qx"] = nc.dram_tensor("qx", [P, NT * (D + QR)], FP8,
                               kind="ExternalInput")
    # xw2 pre-packed host-side: [128, NT, 2W] with the 64 window columns
    # duplicated so dupxw.T @ Wv yields V on both partition halves
    d["d_xw2"] = nc.dram_tensor("xw2", [P, NT * 2 * W], FP16, kind="ExternalInput")
    d["d_wk"] = nc.dram_tensor("wk", [P, NT * D], FP16, kind="ExternalInput")
    d["d_wv"] = nc.dram_tensor("wv", [P, NT * D], FP16, kind="ExternalInput")
    d["d_wo"] = nc.dram_tensor("wo", [P, NT * D], FP16, kind="ExternalInput")
    # cst cols: 0:8 bq*SCALE | 8:16 bk | 16:24 bo | 24:32 (unused) | 32:40 exp bias
    d["d_cst"] = nc.dram_tensor("cst", [P, 5 * NT], F32, kind="ExternalInput")
    # db: block-diag ones for the softmax denominator matmul
    d["d_db"] = nc.dram_tensor("db", [P, P], FP16, kind="ExternalInput")
    # bv broadcast across partitions in the vb block layout [P, NT*P]
    d["d_bvb"] = nc.dram_tensor("bvb", [P, NT * P], FP16, kind="ExternalInput")
    # out: [128, qh*4096 + pair*1024 + 512*odd + q] (two tiles per DMA for
    # 2KB lines); host unpacks
    d["d_out"] = nc.dram_tensor("ot", [P, 2 * NT * 512], FP16,
                                kind="ExternalOutput")

    with tile.TileContext(nc) as tc:
        _emit(nc, tc, d)
    nc.compile()
    return nc


def _emit(nc, tc, d):
    from contextlib import ExitStack

    MM = nc.tensor.matmul
    dma = nc.sync.dma_start
    odma = nc.scalar.dma_start

    with ExitStack() as ctx:
        # ---- persistent SBUF ----
        pers = ctx.enter_context(tc.tile_pool(name="pers", bufs=1))
        t_xw2 = pers.tile([P, NT, 2 * W], FP16, tag="xw2")
        t_kb = pers.tile([P, NT, P], FP16, tag="kb")      # K^T block-diags
        t_vb = pers.tile([P, NT, P], FP16, tag="vb")      # V block-diags
        t_qt = pers.tile([P, NT, QR], FP16, tag="qt")
        t_at = pers.tile([P, NT, QR], FP16, tag="at")
        t_cst = pers.tile([P, 5 * NT], F32, tag="cst")
        t_db = pers.tile([P, P], FP16, tag="db")
        t_bvb = pers.tile([P, NT, P], FP16, tag="bvb")
        t_qx = pers.tile([P, NT, D + QR], FP8, tag="qx")
        # wk/wv/wo are packed column-half-major: [P, g, chunk, 512] so the
        # first DMA covers output columns 0:512 of ALL chunks (what the
        # K/V/O phase g=0 loops consume first)
        t_wk = pers.tile([P, 2, NT, 512], FP16, tag="wk")
        t_wv = pers.tile([P, 2, NT, 512], FP16, tag="wv")
        t_wo = pers.tile([P, 2, NT, 512], FP16, tag="wo")
        t_warm = pers.tile([P, P], FP16, tag="warm")

        t_bq = t_cst[:, 0:NT]
        t_bk = t_cst[:, NT:2 * NT]
        t_bo = t_cst[:, 2 * NT:3 * NT]
        t_eb = t_cst[:, 4 * NT:5 * NT]
        t_ones = t_db[:, 0:P]

        # ---- input DMAs up front, in consumption order.  The first wq/xq
        # chunk pair rides the otherwise-idle scalar queue (its DMA rings
        # start cold in parallel with the sync queue's) so the first
        # DoubleRow wave starts ~1.5us earlier.  Remaining wq/xq quarters
        # (2KB lines) then wk/wv/wo column-halves (8KB lines) on sync. ----
        odma(t_cst[:], d["d_cst"].ap())
        odma(t_db[:], d["d_db"].ap())
        odma(t_bvb[:], d["d_bvb"].ap())
        # the 16 DMA channels are SHARED between the sync and scalar
        # queues, so the critical first chunk pair must lead the sync
        # stream itself
        CW = D + QR
        for q in range(4):
            dma(t_qx[:, 2 * q:2 * q + 2, :],
                d["d_qx"].ap()[:, 2 * q * CW:(2 * q + 2) * CW])
        dma(t_xw2[:], d["d_xw2"].ap())
        GC = NT * 512
        for g in range(2):
            dma(t_wk[:, g, :, :], d["d_wk"].ap()[:, g * GC:(g + 1) * GC])
            dma(t_wv[:, g, :, :], d["d_wv"].ap()[:, g * GC:(g + 1) * GC])
        for g in range(2):
            dma(t_wo[:, g, :, :], d["d_wo"].ap()[:, g * GC:(g + 1) * GC])

        # zero the off-diagonal quadrants of the block tiles + warm tile
        nc.gpsimd.memset(t_warm[:], 0.0)
        nc.gpsimd.memset(t_kb[:], 0.0)
        nc.gpsimd.memset(t_vb[:], 0.0)

        # ---- single PSUM arena: 8 explicit bank-tags (bufs=1 each), so
        # every bank reuse is a per-tag WAR dependency on exactly the
        # intended reader -- no pool-boundary barriers. ----
        arena = ctx.enter_context(tc.tile_pool(name="arena", bufs=1,
                                               space="PSUM"))

        def bank(i, name, shape=(P, 512)):
            return arena.tile(list(shape), F32, tag=f"q{i}", name=name,
                              bufs=1)

        # ---- PE warmup: back-to-back matmuls on the memset tile keep the
        # PE busy during the initial DMA fill so the HAM clock-gate is at
        # 2.4 GHz when the first real matmul lands ----
        wps = bank(7, "wps", (P, P))
        for _ in range(30):
            MM(wps[:], t_warm[:], t_warm[:], start=True, stop=True)

        # GpSimd has no PSUM port: all PSUM-evacuating epilogues alternate
        # between the Scalar and Vector engines.
        epi = [nc.scalar, nc.vector]

        def bias_add(i, out, in_, col):
            e = epi[i % 2]
            if e is nc.scalar:
                e.activation(out, in_, IDENT, bias=col)
            else:
                e.tensor_scalar_add(out, in_, col)

        def q_epi(i, out, in_, col, scalar_only=False):
            # out = in * 2^-8 + col  (undo the fp8 weight scale)
            e = nc.scalar if scalar_only else epi[i % 2]
            if e is nc.scalar:
                e.activation(out, in_, IDENT, bias=col, scale=1.0 / QS)
            else:
                e.tensor_scalar(out, in_, 1.0 / QS, col, MULT, ADD)

        # ---- Q proj qh=0 (chunk-pair-outer over all 8 banks): trails the
        # wq/xq DMA stream pair by pair ----
        qps = [bank(t, f"qp{t}") for t in range(NT)]
        for cp in range(NT // 2):
            for t in range(NT):
                MM(qps[t][:],
                   t_qx[:, 2 * cp:2 * cp + 2, t * P:(t + 1) * P],
                   t_qx[:, 2 * cp:2 * cp + 2, D:D + 512],
                   start=(cp == 0), stop=(cp == NT // 2 - 1),
                   perf_mode=DR)
        # epilogues for banks q6/q7 run FIRST on their engine queues so the
        # qh=1 tiles (which reuse those banks) start without waiting for
        # the whole epilogue drain
        for pos, t in enumerate([6, 7, 0, 1, 2, 3, 4, 5]):
            q_epi(pos, t_qt[:, t, 0:512], qps[t][:], t_bq[:, t:t + 1])

        def qh1_tile(i):
            # Q qh=1 tile i: 4 DR matmuls + scalar-only epilogue (the
            # vector queue is saturated by recip+mul when these run inside
            # the attn(0) interleave)
            qp1 = bank(6 + (i % 2), f"dq{i}")
            for cp in range(NT // 2):
                MM(qp1[:],
                   t_qx[:, 2 * cp:2 * cp + 2, i * P:(i + 1) * P],
                   t_qx[:, 2 * cp:2 * cp + 2, D + 512:D + QR],
                   start=(cp == 0), stop=(cp == NT // 2 - 1),
                   perf_mode=DR)
            q_epi(i, t_qt[:, i, 512:QR], qp1[:], t_bq[:, i:i + 1],
                  scalar_only=True)

        # tiles 0-1 of Q qh=1 fill the PE while the wk g=0 DMA (behind the
        # whole qx stream) lands; the remaining six fill attn(0)'s
        # Vector-bound softmax window below
        qh1_tile(0)
        qh1_tile(1)

        # ---- K^T proj: 4 output tiles share ONE PSUM bank as a single
        # accumulation group.  Group g=1 is interleaved into the V g=0
        # matmul stream: its 32 LDWEIGHTS (~97ns each, the real cost of
        # these N=64 matmuls) pull ahead inside the PE queue while the
        # 512-wide V matmuls stream. ----
        def k_group(g, c):
            kp = k_banks[g]
            for j in range(4):
                MM(kp[:, j * W:(j + 1) * W],
                   t_wk[:, g, c, j * P:(j + 1) * P],
                   t_xw2[:, c, 0:W],
                   start=(c == 0 and j == 0),
                   stop=(c == NT - 1 and j == 3),
                   skip_group_check=True)

        def k_epi(g):
            for j in range(4):
                t = g * 4 + j
                kp = k_banks[g]
                bias_add(t, t_kb[0:64, t, 0:64],
                         kp[0:64, j * W:(j + 1) * W], t_bk[0:64, t:t + 1])
                bias_add(t + 1, t_kb[64:128, t, 64:128],
                         kp[64:128, j * W:(j + 1) * W],
                         t_bk[64:128, t:t + 1])

        k_banks = [bank(0, "kg0", (P, 4 * W)), bank(1, "kg1", (P, 4 * W))]
        for c in range(NT):
            k_group(0, c)
        k_epi(0)

        def v_group(g):
            vp = bank(2 + g, f"vg{g}", (P, 4, P))
            for c in range(NT):
                if g == 1:
                    k_group(1, c)
                MM(vp[:, :, :], t_xw2[:, c, :],
                   t_wv[:, g, c, :],
                   start=(c == 0), stop=(c == NT - 1),
                   skip_group_check=True)
            if g == 1:
                k_epi(1)
            gs = slice(4 * g, 4 * (g + 1))
            nc.vector.tensor_add(t_vb[0:64, gs, 0:64],
                                 vp[0:64, :, 0:64],
                                 t_bvb[0:64, gs, 0:64])
            nc.vector.tensor_add(t_vb[64:128, gs, 64:128],
                                 vp[64:128, :, 64:128],
                                 t_bvb[64:128, gs, 64:128])

        v_group(0)

        # ---- SBUF pools for the attention chain + output staging ----
        ppool = ctx.enter_context(tc.tile_pool(name="pp", bufs=3))
        rpool = ctx.enter_context(tc.tile_pool(name="rp", bufs=2))
        opool = ctx.enter_context(tc.tile_pool(name="op", bufs=4))

        # attention pair chain, split in two halves so the PE-side (scores)
        # can lead the PSUM-side (AV + denominator) by one pair: the AV
        # matmul needs exp(p) which lags ~0.7us on the Scalar queue.
        def attn_head(qg, pr):
            q0 = qg * 512
            ps = bank(4 + (pr % 2), f"sc{qg}_{pr}")
            MM(ps[:], t_kb[:, pr, :], t_qt[:, pr, q0:q0 + 512],
               start=True, stop=True)
            p = ppool.tile([P, 512], FP16, tag="p", name=f"p{qg}_{pr}")
            nc.scalar.activation(p[:], ps[:], EXP, bias=t_eb[:, pr:pr + 1])
            return p

        def attn_tail(qg, pr, p):
            q0 = qg * 512
            pav = bank(2 + (pr % 2), f"av{qg}_{pr}")
            MM(pav[:], t_vb[:, pr, :], p[:], start=True, stop=True)
            pden = bank(pr % 2, f"dn{qg}_{pr}")
            MM(pden[:], t_ones, p[:], start=True, stop=True)
            rec = rpool.tile([P, 512], F32, tag="rec", name=f"rec{qg}_{pr}")
            nc.vector.reciprocal_approx_fast(out=rec[:], in_=pden[:])
            nc.vector.tensor_mul(t_at[:, pr, q0:q0 + 512], pav[:], rec[:])

        def oproj_tile(qh, t, o, ti, banksel, split_epi=False):
            q0 = qh * 512
            ps = bank(banksel, f"of{qh}_{t}")
            g, jj = t // 4, t % 4
            for c in range(NT):
                MM(ps[:], t_wo[:, g, c, jj * P:(jj + 1) * P],
                   t_at[:, c, q0:q0 + 512],
                   start=(c == 0), stop=(c == NT - 1))
            if split_epi:
                # final tiles: halve the epilogue latency by running both
                # engines in parallel on [128,256] halves
                o0 = o[:, ti * 512:ti * 512 + 256]
                o1 = o[:, ti * 512 + 256:(ti + 1) * 512]
                nc.scalar.activation(o0, ps[:, 0:256], IDENT,
                                     bias=t_bo[:, t:t + 1])
                nc.vector.tensor_scalar_add(o1, ps[:, 256:512],
                                            t_bo[:, t:t + 1])
            else:
                bias_add(t, o[:, ti * 512:(ti + 1) * 512], ps[:],
                         t_bo[:, t:t + 1])

        def out_dma(qh, tp, o):
            out_q = dma if tp % 2 == 0 else odma
            col = qh * (NT * 512) + tp * 1024
            out_q(d["d_out"].ap()[:, col:col + 1024], o[:])

        # ---- attn(0) interleaved with Q proj qh=1: the attention chain is
        # Vector-bound (~1.2us/pair vs 0.65us of PE work), so the qh=1
        # DoubleRow matmuls fill the PE while attn(0)'s softmax chain
        # drains.  qh=1 rotates banks q6/q7 (WAR on qh0 epilogues t6/t7);
        # scores lead AV by one pair so exp() latency never stalls the PE.
        prev = None
        for i in range(NT + 1):
            if i == 4:
                # kb/vb blocks 4-7 (column-group 1): their weight DMAs land
                # while attention pairs 0-3 run
                v_group(1)
            if i < NT:
                p = attn_head(0, i)
                if i >= 2:
                    qh1_tile(i)
            if prev is not None:
                attn_tail(0, i - 1, prev)
            prev = p if i < NT else None

        # ---- attn(1) interleaved with O proj qh=0 (same pattern: O-proj
        # matmuls keep the PE busy under attn(1)'s softmax chain).  O tiles
        # rotate banks q6/q7; out DMAs drain per tile pair. ----
        prev = None
        o_tile = None
        for i in range(NT + 1):
            if i < NT:
                p = attn_head(1, i)
                if i % 2 == 0:
                    o_tile = opool.tile([P, 1024], FP16, tag="o",
                                        name=f"o0_{i // 2}")
                oproj_tile(0, i, o_tile, i % 2, 6 + (i % 2))
                if i % 2 == 1:
                    out_dma(0, i // 2, o_tile)
            if prev is not None:
                attn_tail(1, i - 1, prev)
            prev = p if i < NT else None

        # ---- O proj qh=1: pure PE, rotate banks q0..q5 for slack against
        # the epilogue queues ----
        for t in range(NT):
            if t % 2 == 0:
                o_tile = opool.tile([P, 1024], FP16, tag="o",
                                    name=f"o1_{t // 2}")
            oproj_tile(1, t, o_tile, t % 2, t % 6, split_epi=(t >= NT - 2))
            if t >= NT - 2:
                # final pair: drain each tile as soon as its epilogue is
                # done, split across both queues (latency > descriptor
                # efficiency at the very end)
                col = NT * 512 + (t // 2) * 1024 + (t % 2) * 512
                half = (t % 2) * 512
                dma(d["d_out"].ap()[:, col:col + 256],
                    o_tile[:, half:half + 256])
                odma(d["d_out"].ap()[:, col + 256:col + 512],
                     o_tile[:, half + 256:half + 512])
            elif t % 2 == 1:
                out_dma(1, t // 2, o_tile)


def _get_nc():
    if "nc" not in _CACHE:
        _CACHE["nc"] = _build()
    return _CACHE["nc"]


def kernel(x, Wq, bq, Wk, bk, Wv, bv, Wo, bo, mask):
    x = np.asarray(x, np.float32)
    Wq = np.asarray(Wq, np.float32); bq = np.asarray(bq, np.float32)
    Wk = np.asarray(Wk, np.float32); bk = np.asarray(bk, np.float32)
    Wv = np.asarray(Wv, np.float32); bv = np.asarray(bv, np.float32)
    Wo = np.asarray(Wo, np.float32); bo = np.asarray(bo, np.float32)
    mask = np.asarray(mask, np.int32)
    assert x.shape == (B, S, D) and mask.shape == (B, S)

    nc = _get_nc()

    def cvt(a):
        return np.ascontiguousarray(a, dtype=np.float16)

    k_last = np.array([
        (np.nonzero(mask[b])[0][-1] if mask[b].any() else S - 1)
        for b in range(B)
    ])
    win0s = np.maximum(0, k_last + 1 - W)
    slopes = 1.0 / 2.0 ** (np.arange(H, dtype=np.float32) / H)

    def sbuf_pack(a):
        # [D, N] row-major -> [P, NT*N]: row p holds chunks c=0..NT-1 of
        # a[c*P+p, :] concatenated (the chunk-major SBUF tile layout)
        return np.ascontiguousarray(
            a.reshape(NT, P, a.shape[1]).transpose(1, 0, 2).reshape(P, -1))

    def gpack(a):
        # [D, D] -> [P, 2*NT*512] column-half-major: [p, g, c, j] =
        # a[c*P+p, g*512+j]
        return np.ascontiguousarray(
            a.reshape(NT, P, 2, 512).transpose(1, 2, 0, 3).reshape(P, -1))

    wq_t8 = np.clip(Wq.T * (SCALE * QS), -240.0, 240.0).astype(E4NP) \
        .reshape(NT, P, D).transpose(1, 0, 2)  # [P, NT, D]
    wk_t = gpack(cvt(Wk.T))
    wv_t = gpack(cvt(Wv.T))
    wo_t = gpack(cvt(Wo.T))

    db = np.zeros((P, P), np.float16)
    db[0:64, 0:64] = 1.0
    db[64:128, 64:128] = 1.0

    bvb = np.broadcast_to(
        bv.astype(np.float16).reshape(1, NT * P), (P, NT * P))
    bvb = np.ascontiguousarray(bvb)

    cst_common = np.zeros((P, 5 * NT), np.float32)
    cst_common[:, 0:NT] = (bq * SCALE).reshape(NT, P).T
    cst_common[:, NT:2 * NT] = bk.reshape(NT, P).T
    cst_common[:, 2 * NT:3 * NT] = bo.reshape(NT, P).T

    in_maps = []
    for c in range(NCORES):
        b = c // 2
        q0 = (c % 2) * QR
        win0 = int(win0s[b])
        xT = x[b].T  # [D, S]
        kk = win0 + np.arange(W, dtype=np.float32) - float(k_last[b])  # [W]
        pen = np.where(mask[b, win0:win0 + W] == 0, PEN, 0.0).astype(np.float32)
        cst = cst_common.copy()
        for pr in range(NT):
            cst[0:64, 4 * NT + pr] = slopes[2 * pr] * kk + pen
            cst[64:128, 4 * NT + pr] = slopes[2 * pr + 1] * kk + pen
        xw = xT[:, win0:win0 + W].reshape(NT, P, W).transpose(1, 0, 2)  # [P,NT,W]
        xw2 = np.concatenate([xw, xw], axis=2)  # [P, NT, 2W]
        xq8 = np.clip(xT[:, q0:q0 + QR], -240.0, 240.0).astype(E4NP) \
            .reshape(NT, P, QR).transpose(1, 0, 2)  # [P, NT, QR]
        qx8 = np.ascontiguousarray(
            np.concatenate([wq_t8, xq8], axis=2).reshape(P, -1))
        in_maps.append({
            "qx": qx8,
            "xw2": np.ascontiguousarray(
                xw2.reshape(P, NT * 2 * W), dtype=np.float16),
            "wk": wk_t, "wv": wv_t, "wo": wo_t,
            "cst": cst, "db": db, "bvb": bvb,
        })

    global _last_in_maps
    _last_in_maps = in_maps
    res = run_bass_kernel_spmd(nc, in_maps, core_ids=list(range(NCORES)))
    out = np.empty((B, S, D), np.float32)
    for c in range(NCORES):
        b = c // 2
        q0 = (c % 2) * QR
        # ot layout: [p, qh*4096 + tp*1024 + ti*512 + q] for dim-tile
        # t = 2*tp+ti -> out[q0 + qh*512 + q, t*128 + p]
        ot = res.results[c]["ot"].astype(np.float32)
        ot = ot.reshape(P, 2, NT // 2, 2, 512)       # p, qh, tp, ti, q
        out[b, q0:q0 + QR, :] = (
            ot.transpose(1, 4, 2, 3, 0)              # qh, q, tp, ti, p
            .reshape(QR, D))
    return out


if __name__ == "__main__":
    rng = np.random.default_rng(0)
    x = rng.standard_normal((B, S, D), dtype=np.float32)
    w = lambda: (rng.standard_normal((D, D)) * 0.02).astype(np.float32)
    z = np.zeros((D,), np.float32)
    o = kernel(x, w(), z, w(), z, w(), z, w(), z, np.ones((B, S), np.int32))
    print("ran", o.shape, o.dtype)
